# revision 34
# baseline (speedup 1.0000x reference)
"""Trainium2 Bass kernel for a single-head transformer block.

Reference computation (B=4, S=4096, D=1024, fp32):
    h   = rmsnorm(x) * g
    qkv = h @ w_qkv + b_qkv ;  q,k,v = split(qkv)
    q,k = ternary_rope(q), ternary_rope(k)      (cos/sin rounded to {-1,0,1})
    p   = softmax(q@k.T / sqrt(D) * ln3)        (base-3 softmax)
    out = (p @ v) @ w_proj + b_proj + x

Sharding: 8 cores, 2 per batch. Each core computes K/V for its full batch
(4096 keys) and attention for its 2048 query rows. Per-core inputs are
reordered so the core's own query rows come first (attention over keys is
permutation invariant); rope tables are passed per-core in the same order.

All heavy matmuls run in fp8 e4m3 with DoubleRow perf mode (K=256 per
instruction, 2x PE throughput). The attention path contributes ~1% of the
output norm (the fp32 residual dominates), so fp8 keeps rel err ~7e-4.
K^T, Q^T and V live in SBUF for the whole kernel - no DRAM roundtrips.
The unnormalized attention output is scaled by 1/64 before fp8 quantization
(folded back via the softmax-sum reciprocal). NOTE: the PE transpose ignores
the identity operand's values, so scale folds must go elsewhere.
"""

import numpy as np
import ml_dtypes

import concourse.bass as bass
import concourse.tile as tile
from concourse import mybir
from concourse.bass_utils import run_bass_kernel_spmd
from concourse.masks import make_identity

F8 = mybir.dt.float8e4
BF16 = mybir.dt.bfloat16
F32 = mybir.dt.float32
NP_F8 = ml_dtypes.float8_e4m3

B, S, D = 4, 4096, 1024
P = 128
HALF = S // 2          # 2048 query rows per core
N_CORES = 8
RCH = 512              # row chunk for the qkv phase
N_RCH = S // RCH       # 8
N_QCH = HALF // RCH    # 4
NKT = S // P           # 32 key tiles
ND = D // P            # 8 d-slabs
OSCALE = 1.0 / 64.0    # pre-quantization scale for unnormalized attn out
WSCALE = 16.0          # fp8 weight pre-scale (keeps w out of the subnormal
                       # flush-to-zero range); undone in the psum copies

EPS = 1e-6
LN3 = 1.0986122886681098
ROPE_BASE = 10000.0

DR = mybir.MatmulPerfMode.DoubleRow

LAST_RESULT = None     # BassKernelResults of the most recent run (for test.py)


def _split_multiwait(nc, max_waits=1):
    """Walrus in this build rejects instructions carrying many sem waits
    (the Tile end-of-kernel drain has one per engine/queue). Hoist excess
    waits onto single-wait NoOps just before the offending instruction."""
    for fn in nc.m.functions:
        for blk in fn.blocks:
            insts = list(blk.instructions)
            out, changed = [], False
            for ins in insts:
                si = ins.sync_info
                waits = list(si.on_wait) if si is not None and si.on_wait else []
                if len(waits) > max_waits:
                    changed = True
                    for j, w in enumerate(waits[:-max_waits]):
                        out.append(mybir.InstNoOp(
                            name=f"{ins.name}-sw{j}",
                            engine=ins.engine,
                            sync_info=mybir.SyncInfo(on_wait=[w], on_update=[]),
                            bass_nofuse=True,
                        ))
                    ins.sync_info = mybir.SyncInfo(
                        on_wait=waits[-max_waits:],
                        on_update=list(si.on_update) if si.on_update else [])
                out.append(ins)
            if changed:
                blk.instructions = out


def _ternary_tables(S=S):
    """Ternary rope cos/sin half-tables, transposed: [D/2, S] float32."""
    half = D // 2
    inv_freq = (1.0 / (ROPE_BASE ** (np.arange(half, dtype=np.float32) / half))
                ).astype(np.float32)
    ang = np.arange(S, dtype=np.float32)[:, None] * inv_freq[None, :]  # [S, half]
    cos = np.round(np.cos(ang)).astype(np.float32)
    sin = np.round(np.sin(ang)).astype(np.float32)
    return cos.T.copy(), sin.T.copy()  # [half, S]


def _prepare_in_maps(x, g_norm, w_qkv, b_qkv, w_proj, b_proj, S=S):
    HALF = S // 2
    cos_h, sin_h = _ternary_tables(S)
    wqkv_f8 = np.ascontiguousarray(
        (g_norm[:, None] * w_qkv * WSCALE)).astype(NP_F8)
    wp_f8 = np.ascontiguousarray(w_proj * WSCALE).astype(NP_F8)
    in_maps = []
    for c in range(N_CORES):
        b, h = c // 2, c % 2
        own = slice(h * HALF, (h + 1) * HALF)
        other = slice((1 - h) * HALF, (2 - h) * HALF)
        perm = np.concatenate([np.arange(own.start, own.stop),
                               np.arange(other.start, other.stop)])
        xb = x[b]
        xbf = xb.astype(ml_dtypes.bfloat16).astype(np.float32)
        rv = 1.0 / np.sqrt(np.mean(xbf * xbf, axis=-1) + EPS)
        in_maps.append({
            # x^T, column-permuted so own rows come first: [D, S]
            "x_t": np.ascontiguousarray(xb[perm].T).astype(ml_dtypes.bfloat16),
            # per-row 1/rms, host-computed (rmsnorm scale), permuted
            "rv": np.ascontiguousarray(rv[perm][None, :]).astype(
                ml_dtypes.bfloat16),
            "res": np.ascontiguousarray(xb[own] + b_proj[None, :]),
            "wqkv": wqkv_f8,
            "wp": wp_f8,
            "bqkv": b_qkv,
            "cos_t": np.ascontiguousarray(cos_h[:, perm]).astype(ml_dtypes.bfloat16),
            "sin_t": np.ascontiguousarray(sin_h[:, perm]).astype(ml_dtypes.bfloat16),
        })
    return in_maps


def _build(has_bqkv: bool, S=S, ph12=True, ph3=True, split=True, dump=False):
    HALF = S // 2
    N_RCH = S // RCH
    N_QCH = max(HALF // RCH, 1)
    nc = bass.Bass("TRN2", target_bir_lowering=False, debug=False,
                   num_devices=N_CORES)

    x_t = nc.dram_tensor("x_t", [D, S], BF16, kind="ExternalInput").ap()
    res_d = nc.dram_tensor("res", [HALF, D], F32, kind="ExternalInput").ap()
    rv_d = nc.dram_tensor("rv", [1, S], BF16, kind="ExternalInput").ap()
    wqkv_d = nc.dram_tensor("wqkv", [D, 3 * D], F8, kind="ExternalInput").ap()
    wp_d = nc.dram_tensor("wp", [D, D], F8, kind="ExternalInput").ap()
    bqkv_d = nc.dram_tensor("bqkv", [3 * D], F32, kind="ExternalInput").ap()
    cos_d = nc.dram_tensor("cos_t", [D // 2, S], BF16, kind="ExternalInput").ap()
    sin_d = nc.dram_tensor("sin_t", [D // 2, S], BF16, kind="ExternalInput").ap()
    out_d = nc.dram_tensor("out", [HALF, D], F32, kind="ExternalOutput").ap()
    dumps = {}
    if dump:
        dumps["kt"] = nc.dram_tensor("d_kt", [P, ND, S], F8, kind="ExternalOutput").ap()
        dumps["qt"] = nc.dram_tensor("d_qt", [P, ND, HALF], F8, kind="ExternalOutput").ap()
        dumps["v"] = nc.dram_tensor("d_v", [P, NKT, D], F8, kind="ExternalOutput").ap()
        dumps["pt"] = nc.dram_tensor("d_pt", [P, NKT, RCH], F8, kind="ExternalOutput").ap()
        dumps["acc"] = nc.dram_tensor("d_acc", [P, RCH], F32, kind="ExternalOutput").ap()
        dumps["recip"] = nc.dram_tensor("d_recip", [P, RCH // P], F32, kind="ExternalOutput").ap()
        dumps["ot"] = nc.dram_tensor("d_ot", [P, ND, RCH], F8, kind="ExternalOutput").ap()

    x_r = x_t.rearrange("(o p) s -> p o s", p=P)           # [128, 8, 4096]
    wqkv_r = wqkv_d.rearrange("(o p) n -> p o n", p=P)     # [128, 8, 3072]
    wp_r = wp_d.rearrange("(o p) n -> p o n", p=P)         # [128, 8, 1024]
    bqkv_r = bqkv_d.rearrange("(o p) -> p o", p=P)         # [128, 24]
    cos_r = cos_d.rearrange("(o p) s -> p o s", p=P)       # [128, 4, 4096]
    sin_r = sin_d.rearrange("(o p) s -> p o s", p=P)

    with tile.TileContext(nc) as tc:
        with tc.tile_pool(name="singles", bufs=1) as singles:
            ident = singles.tile([P, P], F32)
            make_identity(nc, ident)
            ones8_pad = singles.tile([P, 2, 16], F8)
            nc.vector.memset(ones8_pad, 1.0)
            ones8 = ones8_pad[:, :, 0:1]
            onesc = singles.tile([1, P], BF16)
            nc.vector.memset(onesc, 1.0)
            eps_sb = singles.tile([1, 1], F32)
            nc.vector.memset(eps_sb, EPS)
            wqkv_sb = [singles.tile([P, 2, 3 * D], F8, name=f"wqkv{i}")
                       for i in range(ND // 2)]
            wp_sb = singles.tile([P, ND, D], F8)
            bqkv_sb = singles.tile([P, 24], F32)

            kt_s = singles.tile([P, ND, S], F8)       # rope'd K^T (SBUF-resident)
            qt_s = singles.tile([P, ND, HALF], F8)    # rope'd Q^T
            v_s = singles.tile([P, NKT, D], F8)       # V, keys on partitions

            if ph12:
                _phase12(nc, tc, S, has_bqkv, x_r, wqkv_sb, cos_r, sin_r,
                         bqkv_d, bqkv_sb, ones8, onesc, eps_sb,
                         kt_s, qt_s, v_s, wqkv_r, bqkv_r, rv_d)
            if dump:
                nc.sync.dma_start(dumps["kt"], kt_s)
                nc.sync.dma_start(dumps["qt"], qt_s)
                nc.sync.dma_start(dumps["v"], v_s)
            if ph3:
                _phase3(nc, tc, S, wp_sb, ident, res_d, out_d,
                        kt_s, qt_s, v_s, wp_r, dumps)

    if split:
        _split_multiwait(nc)
    return nc


def _phase12(nc, tc, S, has_bqkv, x_r, wqkv_sb, cos_r, sin_r, bqkv_d, bqkv_sb,
             ones8, onesc, eps_sb, kt_s, qt_s, v_s, wqkv_r, bqkv_r, rv_d):
    """QKV + rope, software-pipelined: the rmsnorm scale chain for the next
    window (ms->sqrt->recip->broadcast->hT) is interleaved into the current
    window's heavy matmul sections so the PE rarely waits on it. Q-chunks
    (which carry two rope passes) are interleaved with K-only chunks so the
    vector engine's backlog drains."""
    N_RCH = S // RCH
    N_QCH = max((S // 2) // RCH, 1)
    with (
        tc.tile_pool(name="xp", bufs=2) as xp,
        tc.tile_pool(name="p12", bufs=2) as p12,
        tc.tile_pool(name="h12", bufs=2) as h12,
        tc.tile_pool(name="tmp12", bufs=2) as tmp12,
        tc.tile_pool(name="st", bufs=1) as st,
        tc.tile_pool(name="ps12", bufs=4, space="PSUM") as ps12,
        tc.tile_pool(name="psms", bufs=2, space="PSUM") as psms,
    ):
        xTs, hTs = {}, {}
        rv_sb = st.tile([1, S], BF16)

        def load_x(j):
            if j is None or j >= N_RCH:
                return
            xT = xp.tile([P, ND, RCH], BF16, tag="xT", name=f"xT{j}")
            rows = slice(j * RCH, (j + 1) * RCH)
            for o in range(ND):
                nc.sync.dma_start(xT[:, o, :], x_r[:, o, rows])
            xTs[j] = xT

        def chain_tail(j):
            # broadcast the host-computed 1/rms row scales, then hT
            # psr (PE broadcast) -> rep (scalar) -> hT (vector, fp8)
            if j is None or j >= N_RCH:
                return
            xT = xTs.pop(j)
            rb = rv_sb[0:1, j * RCH:(j + 1) * RCH]
            psr = psms.tile([P, RCH], F32, tag="psr", name=f"psr{j}")
            nc.tensor.matmul(psr, onesc, rb, start=True, stop=True)
            rep = h12.tile([P, RCH], BF16, tag="rep", name=f"rep{j}")
            nc.scalar.copy(rep, psr)
            hT = h12.tile([P, ND, RCH], F8, tag="hT", name=f"hT{j}")
            for di in range(ND):
                eng = nc.gpsimd if di >= 6 else nc.vector
                eng.tensor_tensor(hT[:, di, :], xT[:, di, :], rep,
                                  mybir.AluOpType.mult)
            hTs[j] = hT

        def qk_mms(r, base, t_qk):
            hT = hTs[r]
            for do in range(ND):
                ps = ps12.tile([P, RCH], F32, tag="ps12")
                for i in range(ND // 2):
                    nc.tensor.matmul(
                        ps,
                        wqkv_sb[i][:, :, base + do * P: base + (do + 1) * P],
                        hT[:, 2 * i:2 * i + 2, :],
                        start=(i == 0), stop=(i == ND // 2 - 1),
                        perf_mode=DR)
                if has_bqkv:
                    nc.scalar.activation(
                        t_qk[:, do, :], ps,
                        mybir.ActivationFunctionType.Identity,
                        scale=1.0 / WSCALE,
                        bias=bqkv_sb[:, base // P + do: base // P + do + 1])
                else:
                    nc.scalar.activation(
                        t_qk[:, do, :], ps,
                        mybir.ActivationFunctionType.Copy,
                        scale=1.0 / WSCALE)

        def rope(r, t_qk, cos_c, sin_c, dst):
            # big-slice ops: all 4 d-block pairs per instruction
            rows = slice(r * RCH, (r + 1) * RCH)
            m1 = tmp12.tile([P, 4, RCH], BF16, tag="m1")
            nc.vector.tensor_tensor(m1, t_qk[:, 0:4, :], cos_c,
                                    mybir.AluOpType.mult)
            m2 = tmp12.tile([P, 4, RCH], BF16, tag="m2")
            nc.vector.tensor_tensor(m2, t_qk[:, 4:8, :], sin_c,
                                    mybir.AluOpType.mult)
            nc.vector.tensor_tensor(dst[:, 0:4, rows], m1, m2,
                                    mybir.AluOpType.subtract)
            m3 = tmp12.tile([P, 4, RCH], BF16, tag="m1")
            nc.vector.tensor_tensor(m3, t_qk[:, 4:8, :], cos_c,
                                    mybir.AluOpType.mult)
            m4 = tmp12.tile([P, 4, RCH], BF16, tag="m2")
            nc.vector.tensor_tensor(m4, t_qk[:, 0:4, :], sin_c,
                                    mybir.AluOpType.mult)
            nc.vector.tensor_tensor(dst[:, 4:8, rows], m3, m4,
                                    mybir.AluOpType.add)

        def v_mms(r):
            hT = hTs[r]
            for sub in range(RCH // P):
                for no in range(D // 512):
                    ps = ps12.tile([P, RCH], F32, tag="ps12")
                    for i in range(ND // 2):
                        nc.tensor.matmul(
                            ps,
                            hT[:, 2 * i:2 * i + 2, sub * P:(sub + 1) * P],
                            wqkv_sb[i][:, :,
                                       2 * D + no * 512: 2 * D + (no + 1) * 512],
                            start=(i == 0), stop=(i == ND // 2 - 1),
                            perf_mode=DR)
                    vdst = v_s[:, r * (RCH // P) + sub, no * 512:(no + 1) * 512]
                    if has_bqkv:
                        vt = tmp12.tile([P, 512], BF16, tag="vtb")
                        nc.scalar.activation(vt, ps,
                                             mybir.ActivationFunctionType.Copy,
                                             scale=1.0 / WSCALE)
                        nc.vector.tensor_tensor(
                            vdst, vt,
                            bass.AP(tensor=bqkv_d.tensor,
                                    offset=bqkv_d.offset + 2 * D + no * 512,
                                    ap=[[0, P], [1, 512]]),
                            mybir.AluOpType.add)
                    else:
                        nc.scalar.activation(vdst, ps,
                                             mybir.ActivationFunctionType.Copy,
                                             scale=1.0 / WSCALE)

        # prologue: x chunk DMAs first so the rmsnorm chain starts
        # immediately; weight slabs follow on other queues
        first = 0
        second = 4 if N_RCH == 8 else 1
        nc.sync.dma_start(rv_sb, rv_d)
        load_x(first)
        for i in range(ND // 2):
            for k in range(2):
                nc.sync.dma_start(wqkv_sb[i][:, k, :], wqkv_r[:, 2 * i + k, :])
        nc.sync.dma_start(bqkv_sb, bqkv_r)
        load_x(second)
        chain_tail(first)

        order = [0, 4, 1, 5, 2, 6, 3, 7] if N_RCH == 8 else list(range(N_RCH))
        for idx, r in enumerate(order):
            rows = slice(r * RCH, (r + 1) * RCH)
            nxt = order[idx + 1] if idx + 1 < len(order) else None
            nx2 = order[idx + 2] if idx + 2 < len(order) else None
            if nx2 is not None:
                load_x(nx2)
            cos_c = p12.tile([P, 4, RCH], BF16, tag="cos")
            nc.sync.dma_start(cos_c, cos_r[:, :, rows])
            sin_c = p12.tile([P, 4, RCH], BF16, tag="sin")
            nc.sync.dma_start(sin_c, sin_r[:, :, rows])

            t_k = p12.tile([P, ND, RCH], BF16, tag="tqk", name=f"tk{r}")
            qk_mms(r, D, t_k)
            rope(r, t_k, cos_c, sin_c, kt_s)
            chain_tail(nxt)
            if r < N_QCH:
                t_q = p12.tile([P, ND, RCH], BF16, tag="tqk", name=f"tq{r}")
                qk_mms(r, 0, t_q)
                rope(r, t_q, cos_c, sin_c, qt_s)
            v_mms(r)
            hTs.pop(r)


def _phase3(nc, tc, S, wp_sb, ident, res_d, out_d, kt_s, qt_s, v_s, wp_r,
            dumps={}):
    N_QCH = max((S // 2) // RCH, 1)
    NKT = S // P
    NSUB = RCH // P
    with (
        tc.tile_pool(name="p3", bufs=2) as p3,
        tc.tile_pool(name="resp", bufs=2) as resp,
        tc.tile_pool(name="outp", bufs=4) as outp,
        tc.tile_pool(name="rcp", bufs=4) as rcp,
        tc.tile_pool(name="ps_s", bufs=2, space="PSUM") as ps_s,
        tc.tile_pool(name="ps_pv", bufs=1, space="PSUM") as ps_pv,
        tc.tile_pool(name="ps_pj", bufs=2, space="PSUM") as ps_pj,
    ):
        nc.sync.dma_start(wp_sb, wp_r)
        for c in range(N_QCH):
            qcols = slice(c * RCH, (c + 1) * RCH)
            rest = resp.tile([P, NSUB, D], F32, tag="res")
            for qs in range(NSUB):
                nc.sync.dma_start(
                    rest[:, qs, :],
                    res_d[c * RCH + qs * P: c * RCH + (qs + 1) * P, :])
            pt = p3.tile([P, NKT, RCH], F8, tag="pt")
            acc = p3.tile([P, RCH], F32, tag="acc")
            recip = rcp.tile([P, NSUB], F32, tag="recip")
            for kt in range(NKT):
                ps = ps_s.tile([P, RCH], F32, tag="ps_s")
                for i in range(ND // 2):
                    nc.tensor.matmul(ps,
                                     kt_s[:, 2 * i:2 * i + 2, kt * P:(kt + 1) * P],
                                     qt_s[:, 2 * i:2 * i + 2, qcols],
                                     start=(i == 0), stop=(i == ND // 2 - 1),
                                     perf_mode=DR)
                nc.scalar.activation(pt[:, kt, :], ps,
                                     mybir.ActivationFunctionType.Exp,
                                     scale=LN3 / 32.0)
                if kt == 0:
                    nc.vector.tensor_copy(acc, pt[:, 0, :])
                else:
                    nc.vector.tensor_tensor(acc, acc, pt[:, kt, :],
                                            mybir.AluOpType.add)
            # per-query softmax sum: transpose + reduce; scale by
            # OSCALE*WSCALE before the reciprocal so o1 = (o@wp)/denom
            for i in range(NSUB):
                pst = ps_s.tile([P, P], F32, tag="ps_s", name=f"pstr{c}_{i}")
                nc.tensor.transpose(pst, acc[:, i * P:(i + 1) * P], ident)
                scol = rcp.tile([P, 1], F32, tag="scol")
                nc.vector.reduce_sum(scol, pst, axis=mybir.AxisListType.X)
                nc.vector.tensor_scalar_mul(scol, scol, OSCALE * WSCALE)
                nc.vector.reciprocal(recip[:, i:i + 1], scol)

            # attn @ V, unnormalized, scaled by 1/64 into fp8
            ot = p3.tile([P, ND, RCH], F8, tag="ot")
            for g in range(2):
                pvs = [ps_pv.tile([P, RCH], F32, tag=f"pv{j}",
                                  name=f"pv{c}_{g}_{j}")
                       for j in range(4)]
                for t in range(NKT // 2):
                    for j in range(4):
                        nc.tensor.matmul(
                            pvs[j],
                            v_s[:, 2 * t:2 * t + 2,
                                g * 512 + j * P: g * 512 + (j + 1) * P],
                            pt[:, 2 * t:2 * t + 2, :],
                            start=(t == 0), stop=(t == NKT // 2 - 1),
                            perf_mode=DR)
                for j in range(4):
                    nc.scalar.activation(ot[:, g * 4 + j, :], pvs[j],
                                         mybir.ActivationFunctionType.Copy,
                                         scale=OSCALE)

            if dumps and c == 0:
                nc.sync.dma_start(dumps["pt"], pt)
                nc.sync.dma_start(dumps["acc"], acc)
                nc.sync.dma_start(dumps["recip"], recip)
                nc.sync.dma_start(dumps["ot"], ot)
            # out = (ot @ wp) * (64/sum) + res
            for qs in range(NSUB):
                for no in range(D // 512):
                    ps = ps_pj.tile([P, 512], F32, tag="pj")
                    for i in range(ND // 2):
                        nc.tensor.matmul(
                            ps, ot[:, 2 * i:2 * i + 2, qs * P:(qs + 1) * P],
                            wp_sb[:, 2 * i:2 * i + 2, no * 512:(no + 1) * 512],
                            start=(i == 0), stop=(i == ND // 2 - 1),
                            perf_mode=DR)
                    o1 = outp.tile([P, 512], F32, tag="o1")
                    nc.vector.tensor_scalar_mul(o1, ps, recip[:, qs:qs + 1])
                    row0 = c * RCH + qs * P
                    o2 = outp.tile([P, 512], F32, tag="o2")
                    nc.vector.tensor_tensor(
                        o2, o1, rest[:, qs, no * 512:(no + 1) * 512],
                        mybir.AluOpType.add)
                    nc.sync.dma_start(
                        out_d[row0:row0 + P, no * 512:(no + 1) * 512], o2)


_CACHED = {}


def kernel(x, g_norm, w_qkv, b_qkv, w_proj, b_proj):
    global LAST_RESULT
    x = np.asarray(x, dtype=np.float32)
    g_norm = np.asarray(g_norm, dtype=np.float32)
    w_qkv = np.asarray(w_qkv, dtype=np.float32)
    b_qkv = np.asarray(b_qkv, dtype=np.float32)
    w_proj = np.asarray(w_proj, dtype=np.float32)
    b_proj = np.asarray(b_proj, dtype=np.float32)

    has_bqkv = bool(np.any(b_qkv))
    key = ("nc", has_bqkv)
    if key not in _CACHED:
        _CACHED[key] = _build(has_bqkv)
    nc = _CACHED[key]

    in_maps = _prepare_in_maps(x, g_norm, w_qkv, b_qkv, w_proj, b_proj)
    LAST_RESULT = run_bass_kernel_spmd(nc, in_maps, list(range(N_CORES)),
                                       trace=False)
    out = np.empty((B, S, D), dtype=np.float32)
    for c in range(N_CORES):
        b, h = c // 2, c % 2
        out[b, h * HALF:(h + 1) * HALF, :] = LAST_RESULT.results[c]["out"]
    return out


# revision 35
# speedup vs baseline: 1.2035x; 1.2035x over previous
"""Trainium2 Bass kernel for a single-head transformer block.

Reference computation (B=4, S=4096, D=1024, fp32):
    h   = rmsnorm(x) * g
    qkv = h @ w_qkv + b_qkv ;  q,k,v = split(qkv)
    q,k = ternary_rope(q), ternary_rope(k)      (cos/sin rounded to {-1,0,1})
    p   = softmax(q@k.T / sqrt(D) * ln3)        (base-3 softmax)
    out = (p @ v) @ w_proj + b_proj + x

Sharding: 8 cores, 2 per batch. Each core computes K/V for its full batch
(4096 keys) and attention for its 2048 query rows. Per-core inputs are
reordered so the core's own query rows come first (attention over keys is
permutation invariant); rope tables are passed per-core in the same order.

All heavy matmuls run in fp8 e4m3 with DoubleRow perf mode (K=256 per
instruction, 2x PE throughput). The attention path contributes ~1% of the
output norm (the fp32 residual dominates), so fp8 keeps rel err ~7e-4.
K^T, Q^T and V live in SBUF for the whole kernel - no DRAM roundtrips.
The unnormalized attention output is scaled by 1/64 before fp8 quantization
(folded back via the softmax-sum reciprocal). NOTE: the PE transpose ignores
the identity operand's values, so scale folds must go elsewhere.
"""

import numpy as np
import ml_dtypes

import concourse.bass as bass
import concourse.tile as tile
from concourse import mybir
from concourse.bass_utils import run_bass_kernel_spmd
from concourse.masks import make_identity

F8 = mybir.dt.float8e4
BF16 = mybir.dt.bfloat16
F32 = mybir.dt.float32
NP_F8 = ml_dtypes.float8_e4m3

B, S, D = 4, 4096, 1024
P = 128
HALF = S // 2          # 2048 query rows per core
N_CORES = 8
RCH = 512              # row chunk for the qkv phase
N_RCH = S // RCH       # 8
N_QCH = HALF // RCH    # 4
NKT = S // P           # 32 key tiles
ND = D // P            # 8 d-slabs
OSCALE = 1.0 / 64.0    # pre-quantization scale for unnormalized attn out
WSCALE = 16.0          # fp8 weight pre-scale (keeps w out of the subnormal
                       # flush-to-zero range); undone in the psum copies

EPS = 1e-6
LN3 = 1.0986122886681098
ROPE_BASE = 10000.0

DR = mybir.MatmulPerfMode.DoubleRow

LAST_RESULT = None     # BassKernelResults of the most recent run (for test.py)


def _split_multiwait(nc, max_waits=1):
    """Walrus in this build rejects instructions carrying many sem waits
    (the Tile end-of-kernel drain has one per engine/queue). Hoist excess
    waits onto single-wait NoOps just before the offending instruction."""
    for fn in nc.m.functions:
        for blk in fn.blocks:
            insts = list(blk.instructions)
            out, changed = [], False
            for ins in insts:
                si = ins.sync_info
                waits = list(si.on_wait) if si is not None and si.on_wait else []
                if len(waits) > max_waits:
                    changed = True
                    for j, w in enumerate(waits[:-max_waits]):
                        out.append(mybir.InstNoOp(
                            name=f"{ins.name}-sw{j}",
                            engine=ins.engine,
                            sync_info=mybir.SyncInfo(on_wait=[w], on_update=[]),
                            bass_nofuse=True,
                        ))
                    ins.sync_info = mybir.SyncInfo(
                        on_wait=waits[-max_waits:],
                        on_update=list(si.on_update) if si.on_update else [])
                out.append(ins)
            if changed:
                blk.instructions = out


def _ternary_tables(S=S):
    """Ternary rope cos/sin half-tables, transposed: [D/2, S] float32."""
    half = D // 2
    inv_freq = (1.0 / (ROPE_BASE ** (np.arange(half, dtype=np.float32) / half))
                ).astype(np.float32)
    ang = np.arange(S, dtype=np.float32)[:, None] * inv_freq[None, :]  # [S, half]
    cos = np.round(np.cos(ang)).astype(np.float32)
    sin = np.round(np.sin(ang)).astype(np.float32)
    return cos.T.copy(), sin.T.copy()  # [half, S]


def _prepare_in_maps(x, g_norm, w_qkv, b_qkv, w_proj, b_proj, S=S):
    HALF = S // 2
    cos_h, sin_h = _ternary_tables(S)
    wqkv_f8 = np.ascontiguousarray(
        (g_norm[:, None] * w_qkv * WSCALE)).astype(NP_F8)
    wp_f8 = np.ascontiguousarray(w_proj * WSCALE).astype(NP_F8)
    in_maps = []
    for c in range(N_CORES):
        b, h = c // 2, c % 2
        own = slice(h * HALF, (h + 1) * HALF)
        other = slice((1 - h) * HALF, (2 - h) * HALF)
        perm = np.concatenate([np.arange(own.start, own.stop),
                               np.arange(other.start, other.stop)])
        xb = x[b]
        xbf = xb.astype(ml_dtypes.bfloat16).astype(np.float32)
        rv = 1.0 / np.sqrt(np.mean(xbf * xbf, axis=-1) + EPS)
        in_maps.append({
            # x^T, column-permuted so own rows come first: [D, S]
            "x_t": np.ascontiguousarray(xb[perm].T).astype(ml_dtypes.bfloat16),
            # per-row 1/rms, host-computed (rmsnorm scale), permuted
            "rv": np.ascontiguousarray(rv[perm][None, :]).astype(
                ml_dtypes.bfloat16),
            "res": np.ascontiguousarray(xb[own] + b_proj[None, :]),
            "wqkv": wqkv_f8,
            "wp": wp_f8,
            "bqkv": b_qkv,
            "cos_t": np.ascontiguousarray(cos_h[:, perm]).astype(ml_dtypes.bfloat16),
            "sin_t": np.ascontiguousarray(sin_h[:, perm]).astype(ml_dtypes.bfloat16),
        })
    return in_maps


def _build(has_bqkv: bool, S=S, ph12=True, ph3=True, split=True, dump=False):
    HALF = S // 2
    N_RCH = S // RCH
    N_QCH = max(HALF // RCH, 1)
    nc = bass.Bass("TRN2", target_bir_lowering=False, debug=False,
                   num_devices=N_CORES)

    x_t = nc.dram_tensor("x_t", [D, S], BF16, kind="ExternalInput").ap()
    res_d = nc.dram_tensor("res", [HALF, D], F32, kind="ExternalInput").ap()
    rv_d = nc.dram_tensor("rv", [1, S], BF16, kind="ExternalInput").ap()
    wqkv_d = nc.dram_tensor("wqkv", [D, 3 * D], F8, kind="ExternalInput").ap()
    wp_d = nc.dram_tensor("wp", [D, D], F8, kind="ExternalInput").ap()
    bqkv_d = nc.dram_tensor("bqkv", [3 * D], F32, kind="ExternalInput").ap()
    cos_d = nc.dram_tensor("cos_t", [D // 2, S], BF16, kind="ExternalInput").ap()
    sin_d = nc.dram_tensor("sin_t", [D // 2, S], BF16, kind="ExternalInput").ap()
    out_d = nc.dram_tensor("out", [HALF, D], F32, kind="ExternalOutput").ap()
    dumps = {}
    if dump:
        dumps["kt"] = nc.dram_tensor("d_kt", [P, ND, S], F8, kind="ExternalOutput").ap()
        dumps["qt"] = nc.dram_tensor("d_qt", [P, ND, HALF], F8, kind="ExternalOutput").ap()
        dumps["v"] = nc.dram_tensor("d_v", [P, NKT, D], F8, kind="ExternalOutput").ap()
        dumps["pt"] = nc.dram_tensor("d_pt", [P, NKT, RCH], F8, kind="ExternalOutput").ap()
        dumps["acc"] = nc.dram_tensor("d_acc", [P, RCH], F32, kind="ExternalOutput").ap()
        dumps["recip"] = nc.dram_tensor("d_recip", [P, RCH // P], F32, kind="ExternalOutput").ap()
        dumps["ot"] = nc.dram_tensor("d_ot", [P, ND, RCH], F8, kind="ExternalOutput").ap()

    x_r = x_t.rearrange("(o p) s -> p o s", p=P)           # [128, 8, 4096]
    wqkv_r = wqkv_d.rearrange("(o p) n -> p o n", p=P)     # [128, 8, 3072]
    wp_r = wp_d.rearrange("(o p) n -> p o n", p=P)         # [128, 8, 1024]
    bqkv_r = bqkv_d.rearrange("(o p) -> p o", p=P)         # [128, 24]
    cos_r = cos_d.rearrange("(o p) s -> p o s", p=P)       # [128, 4, 4096]
    sin_r = sin_d.rearrange("(o p) s -> p o s", p=P)

    with tile.TileContext(nc) as tc:
        with tc.tile_pool(name="singles", bufs=1) as singles:
            ident = singles.tile([P, P], F32)
            make_identity(nc, ident)
            ones8_pad = singles.tile([P, 2, 16], F8)
            nc.vector.memset(ones8_pad, 1.0)
            ones8 = ones8_pad[:, :, 0:1]
            onesc = singles.tile([1, P], BF16)
            nc.vector.memset(onesc, 1.0)
            eps_sb = singles.tile([1, 1], F32)
            nc.vector.memset(eps_sb, EPS)
            wqkv_sb = singles.tile([P, ND, 3 * D], F8)
            wp_sb = singles.tile([P, ND, D], F8)
            bqkv_sb = singles.tile([P, 24], F32)

            kt_s = singles.tile([P, ND, S], F8)       # rope'd K^T (SBUF-resident)
            qt_s = singles.tile([P, ND, HALF], F8)    # rope'd Q^T
            v_s = singles.tile([P, NKT, D], F8)       # V, keys on partitions

            if ph12:
                _phase12(nc, tc, S, has_bqkv, x_r, wqkv_sb, cos_r, sin_r,
                         bqkv_d, bqkv_sb, ones8, onesc, eps_sb,
                         kt_s, qt_s, v_s, wqkv_r, bqkv_r, rv_d)
            if dump:
                nc.sync.dma_start(dumps["kt"], kt_s)
                nc.sync.dma_start(dumps["qt"], qt_s)
                nc.sync.dma_start(dumps["v"], v_s)
            if ph3:
                _phase3(nc, tc, S, wp_sb, ident, res_d, out_d,
                        kt_s, qt_s, v_s, wp_r, dumps)

    if split:
        _split_multiwait(nc)
    return nc


def _phase12(nc, tc, S, has_bqkv, x_r, wqkv_sb, cos_r, sin_r, bqkv_d, bqkv_sb,
             ones8, onesc, eps_sb, kt_s, qt_s, v_s, wqkv_r, bqkv_r, rv_d):
    """QKV + rope, software-pipelined: the rmsnorm scale chain for the next
    window (ms->sqrt->recip->broadcast->hT) is interleaved into the current
    window's heavy matmul sections so the PE rarely waits on it. Q-chunks
    (which carry two rope passes) are interleaved with K-only chunks so the
    vector engine's backlog drains."""
    N_RCH = S // RCH
    N_QCH = max((S // 2) // RCH, 1)
    with (
        tc.tile_pool(name="xp", bufs=2) as xp,
        tc.tile_pool(name="p12", bufs=2) as p12,
        tc.tile_pool(name="h12", bufs=2) as h12,
        tc.tile_pool(name="tmp12", bufs=2) as tmp12,
        tc.tile_pool(name="st", bufs=1) as st,
        tc.tile_pool(name="ps12", bufs=4, space="PSUM") as ps12,
        tc.tile_pool(name="psms", bufs=2, space="PSUM") as psms,
    ):
        xTs, hTs = {}, {}
        rv_sb = st.tile([1, S], BF16)

        def load_x(j):
            if j is None or j >= N_RCH:
                return
            xT = xp.tile([P, ND, RCH], BF16, tag="xT", name=f"xT{j}")
            rows = slice(j * RCH, (j + 1) * RCH)
            for o in range(ND):
                nc.sync.dma_start(xT[:, o, :], x_r[:, o, rows])
            xTs[j] = xT

        def chain_tail(j):
            # broadcast the host-computed 1/rms row scales, then hT
            # psr (PE broadcast) -> rep (scalar) -> hT (vector, fp8)
            if j is None or j >= N_RCH:
                return
            xT = xTs.pop(j)
            rb = rv_sb[0:1, j * RCH:(j + 1) * RCH]
            psr = psms.tile([P, RCH], F32, tag="psr", name=f"psr{j}")
            nc.tensor.matmul(psr, onesc, rb, start=True, stop=True)
            rep = h12.tile([P, RCH], BF16, tag="rep", name=f"rep{j}")
            nc.scalar.copy(rep, psr)
            hT = h12.tile([P, ND, RCH], F8, tag="hT", name=f"hT{j}")
            for di in range(ND):
                nc.vector.tensor_tensor(hT[:, di, :], xT[:, di, :], rep,
                                        mybir.AluOpType.mult)
            hTs[j] = hT

        def qk_mms(r, base, t_qk):
            hT = hTs[r]
            for do in range(ND):
                ps = ps12.tile([P, RCH], F32, tag="ps12")
                for i in range(ND // 2):
                    nc.tensor.matmul(
                        ps,
                        wqkv_sb[:, 2 * i:2 * i + 2,
                                base + do * P: base + (do + 1) * P],
                        hT[:, 2 * i:2 * i + 2, :],
                        start=(i == 0), stop=(i == ND // 2 - 1),
                        perf_mode=DR)
                if has_bqkv:
                    nc.scalar.activation(
                        t_qk[:, do, :], ps,
                        mybir.ActivationFunctionType.Identity,
                        scale=1.0 / WSCALE,
                        bias=bqkv_sb[:, base // P + do: base // P + do + 1])
                else:
                    nc.scalar.activation(
                        t_qk[:, do, :], ps,
                        mybir.ActivationFunctionType.Copy,
                        scale=1.0 / WSCALE)

        def rope(r, t_qk, cos_c, sin_c, dst):
            # big-slice ops: all 4 d-block pairs per instruction
            rows = slice(r * RCH, (r + 1) * RCH)
            m1 = tmp12.tile([P, 4, RCH], BF16, tag="m1")
            nc.vector.tensor_tensor(m1, t_qk[:, 0:4, :], cos_c,
                                    mybir.AluOpType.mult)
            m2 = tmp12.tile([P, 4, RCH], BF16, tag="m2")
            nc.vector.tensor_tensor(m2, t_qk[:, 4:8, :], sin_c,
                                    mybir.AluOpType.mult)
            nc.vector.tensor_tensor(dst[:, 0:4, rows], m1, m2,
                                    mybir.AluOpType.subtract)
            m3 = tmp12.tile([P, 4, RCH], BF16, tag="m1")
            nc.vector.tensor_tensor(m3, t_qk[:, 4:8, :], cos_c,
                                    mybir.AluOpType.mult)
            m4 = tmp12.tile([P, 4, RCH], BF16, tag="m2")
            nc.vector.tensor_tensor(m4, t_qk[:, 0:4, :], sin_c,
                                    mybir.AluOpType.mult)
            nc.vector.tensor_tensor(dst[:, 4:8, rows], m3, m4,
                                    mybir.AluOpType.add)

        def v_mms(r):
            hT = hTs[r]
            for sub in range(RCH // P):
                for no in range(D // 512):
                    ps = ps12.tile([P, RCH], F32, tag="ps12")
                    for i in range(ND // 2):
                        nc.tensor.matmul(
                            ps,
                            hT[:, 2 * i:2 * i + 2, sub * P:(sub + 1) * P],
                            wqkv_sb[:, 2 * i:2 * i + 2,
                                    2 * D + no * 512: 2 * D + (no + 1) * 512],
                            start=(i == 0), stop=(i == ND // 2 - 1),
                            perf_mode=DR)
                    vdst = v_s[:, r * (RCH // P) + sub, no * 512:(no + 1) * 512]
                    if has_bqkv:
                        vt = tmp12.tile([P, 512], BF16, tag="vtb")
                        nc.scalar.activation(vt, ps,
                                             mybir.ActivationFunctionType.Copy,
                                             scale=1.0 / WSCALE)
                        nc.vector.tensor_tensor(
                            vdst, vt,
                            bass.AP(tensor=bqkv_d.tensor,
                                    offset=bqkv_d.offset + 2 * D + no * 512,
                                    ap=[[0, P], [1, 512]]),
                            mybir.AluOpType.add)
                    else:
                        nc.scalar.activation(vdst, ps,
                                             mybir.ActivationFunctionType.Copy,
                                             scale=1.0 / WSCALE)

        # prologue: x chunk DMAs first so the rmsnorm chain starts
        # immediately; weight slabs follow on other queues
        first = 0
        second = 4 if N_RCH == 8 else 1
        nc.sync.dma_start(rv_sb, rv_d)
        load_x(first)
        for o in range(ND):
            nc.sync.dma_start(wqkv_sb[:, o, :], wqkv_r[:, o, :])
        nc.sync.dma_start(bqkv_sb, bqkv_r)
        load_x(second)
        chain_tail(first)

        order = [0, 4, 1, 5, 2, 6, 3, 7] if N_RCH == 8 else list(range(N_RCH))
        for idx, r in enumerate(order):
            rows = slice(r * RCH, (r + 1) * RCH)
            nxt = order[idx + 1] if idx + 1 < len(order) else None
            nx2 = order[idx + 2] if idx + 2 < len(order) else None
            if nx2 is not None:
                load_x(nx2)
            cos_c = p12.tile([P, 4, RCH], BF16, tag="cos")
            nc.sync.dma_start(cos_c, cos_r[:, :, rows])
            sin_c = p12.tile([P, 4, RCH], BF16, tag="sin")
            nc.sync.dma_start(sin_c, sin_r[:, :, rows])

            t_k = p12.tile([P, ND, RCH], BF16, tag="tqk", name=f"tk{r}")
            qk_mms(r, D, t_k)
            rope(r, t_k, cos_c, sin_c, kt_s)
            chain_tail(nxt)
            if r < N_QCH:
                t_q = p12.tile([P, ND, RCH], BF16, tag="tqk", name=f"tq{r}")
                qk_mms(r, 0, t_q)
                rope(r, t_q, cos_c, sin_c, qt_s)
            v_mms(r)
            hTs.pop(r)


def _phase3(nc, tc, S, wp_sb, ident, res_d, out_d, kt_s, qt_s, v_s, wp_r,
            dumps={}):
    N_QCH = max((S // 2) // RCH, 1)
    NKT = S // P
    NSUB = RCH // P
    with (
        tc.tile_pool(name="p3", bufs=2) as p3,
        tc.tile_pool(name="resp", bufs=2) as resp,
        tc.tile_pool(name="outp", bufs=4) as outp,
        tc.tile_pool(name="rcp", bufs=4) as rcp,
        tc.tile_pool(name="ps_s", bufs=2, space="PSUM") as ps_s,
        tc.tile_pool(name="ps_pv", bufs=1, space="PSUM") as ps_pv,
        tc.tile_pool(name="ps_pj", bufs=2, space="PSUM") as ps_pj,
    ):
        nc.sync.dma_start(wp_sb, wp_r)
        for c in range(N_QCH):
            qcols = slice(c * RCH, (c + 1) * RCH)
            rest = resp.tile([P, NSUB, D], F32, tag="res")
            for qs in range(NSUB):
                nc.sync.dma_start(
                    rest[:, qs, :],
                    res_d[c * RCH + qs * P: c * RCH + (qs + 1) * P, :])
            pt = p3.tile([P, NKT, RCH], F8, tag="pt")
            acc = p3.tile([P, RCH], F32, tag="acc")
            recip = rcp.tile([P, NSUB], F32, tag="recip")
            for kt in range(NKT):
                ps = ps_s.tile([P, RCH], F32, tag="ps_s")
                for i in range(ND // 2):
                    nc.tensor.matmul(ps,
                                     kt_s[:, 2 * i:2 * i + 2, kt * P:(kt + 1) * P],
                                     qt_s[:, 2 * i:2 * i + 2, qcols],
                                     start=(i == 0), stop=(i == ND // 2 - 1),
                                     perf_mode=DR)
                nc.scalar.activation(pt[:, kt, :], ps,
                                     mybir.ActivationFunctionType.Exp,
                                     scale=LN3 / 32.0)
                if kt == 0:
                    nc.vector.tensor_copy(acc, pt[:, 0, :])
                else:
                    nc.vector.tensor_tensor(acc, acc, pt[:, kt, :],
                                            mybir.AluOpType.add)
            # per-query softmax sum: transpose + reduce; scale by
            # OSCALE*WSCALE before the reciprocal so o1 = (o@wp)/denom
            for i in range(NSUB):
                pst = ps_s.tile([P, P], F32, tag="ps_s", name=f"pstr{c}_{i}")
                nc.tensor.transpose(pst, acc[:, i * P:(i + 1) * P], ident)
                scol = rcp.tile([P, 1], F32, tag="scol")
                nc.vector.reduce_sum(scol, pst, axis=mybir.AxisListType.X)
                nc.vector.tensor_scalar_mul(scol, scol, OSCALE * WSCALE)
                nc.vector.reciprocal(recip[:, i:i + 1], scol)

            # attn @ V, unnormalized, scaled by 1/64 into fp8
            ot = p3.tile([P, ND, RCH], F8, tag="ot")
            for g in range(2):
                pvs = [ps_pv.tile([P, RCH], F32, tag=f"pv{j}",
                                  name=f"pv{c}_{g}_{j}")
                       for j in range(4)]
                for t in range(NKT // 2):
                    for j in range(4):
                        nc.tensor.matmul(
                            pvs[j],
                            v_s[:, 2 * t:2 * t + 2,
                                g * 512 + j * P: g * 512 + (j + 1) * P],
                            pt[:, 2 * t:2 * t + 2, :],
                            start=(t == 0), stop=(t == NKT // 2 - 1),
                            perf_mode=DR)
                for j in range(4):
                    nc.scalar.activation(ot[:, g * 4 + j, :], pvs[j],
                                         mybir.ActivationFunctionType.Copy,
                                         scale=OSCALE)

            if dumps and c == 0:
                nc.sync.dma_start(dumps["pt"], pt)
                nc.sync.dma_start(dumps["acc"], acc)
                nc.sync.dma_start(dumps["recip"], recip)
                nc.sync.dma_start(dumps["ot"], ot)
            # out = (ot @ wp) * (64/sum) + res
            for qs in range(NSUB):
                for no in range(D // 512):
                    ps = ps_pj.tile([P, 512], F32, tag="pj")
                    for i in range(ND // 2):
                        nc.tensor.matmul(
                            ps, ot[:, 2 * i:2 * i + 2, qs * P:(qs + 1) * P],
                            wp_sb[:, 2 * i:2 * i + 2, no * 512:(no + 1) * 512],
                            start=(i == 0), stop=(i == ND // 2 - 1),
                            perf_mode=DR)
                    o1 = outp.tile([P, 512], F32, tag="o1")
                    nc.vector.tensor_scalar_mul(o1, ps, recip[:, qs:qs + 1])
                    row0 = c * RCH + qs * P
                    o2 = outp.tile([P, 512], F32, tag="o2")
                    nc.vector.tensor_tensor(
                        o2, o1, rest[:, qs, no * 512:(no + 1) * 512],
                        mybir.AluOpType.add)
                    nc.sync.dma_start(
                        out_d[row0:row0 + P, no * 512:(no + 1) * 512], o2)


_CACHED = {}


def kernel(x, g_norm, w_qkv, b_qkv, w_proj, b_proj):
    global LAST_RESULT
    x = np.asarray(x, dtype=np.float32)
    g_norm = np.asarray(g_norm, dtype=np.float32)
    w_qkv = np.asarray(w_qkv, dtype=np.float32)
    b_qkv = np.asarray(b_qkv, dtype=np.float32)
    w_proj = np.asarray(w_proj, dtype=np.float32)
    b_proj = np.asarray(b_proj, dtype=np.float32)

    has_bqkv = bool(np.any(b_qkv))
    key = ("nc", has_bqkv)
    if key not in _CACHED:
        _CACHED[key] = _build(has_bqkv)
    nc = _CACHED[key]

    in_maps = _prepare_in_maps(x, g_norm, w_qkv, b_qkv, w_proj, b_proj)
    LAST_RESULT = run_bass_kernel_spmd(nc, in_maps, list(range(N_CORES)),
                                       trace=False)
    out = np.empty((B, S, D), dtype=np.float32)
    for c in range(N_CORES):
        b, h = c // 2, c % 2
        out[b, h * HALF:(h + 1) * HALF, :] = LAST_RESULT.results[c]["out"]
    return out
